# revision 10
# baseline (speedup 1.0000x reference)
"""Trainium2 Bass kernel for ExpSSGL encoder (3-layer SpMM + signed-noise perturbation).

Strategy (8 NeuronCores):
  - Row-range sharding: core c owns output rows [c*R, (c+1)*R), R = N/8.
  - Edges are bucketed by (128-row destination block, 30000-node source
    bucket); each 128-edge tile is segment-summed into its block via a
    val-weighted one-hot selection matrix (tensor_scalar is_equal*mult)
    matmul accumulated in PSUM.
  - x[col] rows (256B) are fetched with gpsimd dma_gather (SWDGE, int16
    in-bucket indices, <=4096 indices per call, single_packet=False).
  - After each layer, per-core updated rows are AllGather'd so every core
    has the full [N, 64] matrix for the next layer's gather.
  - layer_sum stays resident in SBUF; output = layer_sum / L.
"""
import numpy as np
import ml_dtypes
from contextlib import ExitStack

from concourse import bass, bacc, mybir, tile
from concourse.bass_utils import run_bass_kernel_spmd

P = 128
D = 64
C = 8            # cores
EPS = 0.1
NORM_EPS = 1e-12

f32 = mybir.dt.float32
bf16 = mybir.dt.bfloat16
i16 = mybir.dt.int16

BSZ = 30000      # node bucket size (int16 index range)
SCALL = 32       # max tiles per dma_gather call (4096 indices)
JBLK = 6         # blocks per write group (PSUM has 8 banks; keep 2 spare)


# --------------------------------------------------------------------------
# Host-side data prep
# --------------------------------------------------------------------------

def build_structure(row, col, val, N, n_cores=C, use_bf16=False, bsz=BSZ,
                    jblk=JBLK, scall=SCALL):
    """Pack edges per core into the shared (wg, bucket, block) tile layout."""
    R = N // n_cores
    n_blocks = (R + P - 1) // P
    nbuck = (N + bsz - 1) // bsz
    n_wg = (n_blocks + jblk - 1) // jblk
    np_mdt = ml_dtypes.bfloat16 if use_bf16 else np.float32

    per_core = []
    cnts = np.zeros((n_cores, n_blocks, nbuck), dtype=np.int64)
    for c in range(n_cores):
        sel = (row >= c * R) & (row < (c + 1) * R)
        r = (row[sel] - c * R).astype(np.int64)
        cc = col[sel].astype(np.int64)
        vv = val[sel].astype(np.float32)
        b = r // P
        u = cc // bsz
        cell = b * nbuck + u
        o = np.argsort(cell, kind="stable")
        r, cc, vv, cell = r[o], cc[o], vv[o], cell[o]
        cnts[c] = np.bincount(cell, minlength=n_blocks * nbuck).reshape(
            n_blocks, nbuck)
        per_core.append((r, cc, vv))

    M_bu = ((cnts.max(axis=0) + P - 1) // P).astype(np.int64)   # [n_blocks, nbuck]
    empty = M_bu.sum(axis=1) == 0
    M_bu[empty, 0] = 1

    # tile order: (wg, u, b); calls chunk <= scall tiles within (wg, u)
    tiles_b = []          # block id per tile
    tiles_u = []          # bucket id per tile
    calls = []            # (u, t_start, t_end)
    wg_tile_range = []    # (t_start, t_end) per wg
    t = 0
    for wg in range(n_wg):
        b0, b1 = wg * jblk, min((wg + 1) * jblk, n_blocks)
        wg_start = t
        for u in range(nbuck):
            seg_start = t
            for b in range(b0, b1):
                for _ in range(M_bu[b, u]):
                    tiles_b.append(b)
                    tiles_u.append(u)
                    t += 1
            for cs in range(seg_start, t, scall):
                calls.append((u, cs, min(cs + scall, t)))
        wg_tile_range.append((wg_start, t))
    T = t
    tiles_b = np.array(tiles_b)
    tiles_u = np.array(tiles_u)

    # slot ranges per (b, u) cell in tile order
    cell_slot_start = np.zeros((n_blocks, nbuck), dtype=np.int64)
    pos = np.zeros(1, dtype=np.int64)
    tile_cnt = 0
    for wg in range(n_wg):
        b0, b1 = wg * jblk, min((wg + 1) * jblk, n_blocks)
        for u in range(nbuck):
            for b in range(b0, b1):
                cell_slot_start[b, u] = tile_cnt * P
                tile_cnt += M_bu[b, u]
    assert tile_cnt == T

    first_tile = np.full(n_blocks, -1, dtype=np.int64)
    last_tile = np.zeros(n_blocks, dtype=np.int64)
    for ti in range(T):
        b = tiles_b[ti]
        if first_tile[b] < 0:
            first_tile[b] = ti
        last_tile[b] = ti

    packed = []
    for c in range(n_cores):
        r, cc, vv = per_core[c]
        cell = (r // P) * nbuck + (cc // bsz)
        starts = np.concatenate(
            [[0], np.cumsum(np.bincount(cell, minlength=n_blocks * nbuck))])
        colp = np.zeros(T * P, dtype=np.int64)
        rowp = np.zeros(T * P, dtype=np.float32)
        valp = np.zeros(T * P, dtype=np.float32)
        for b in range(n_blocks):
            for u in range(nbuck):
                e0, e1 = starts[b * nbuck + u], starts[b * nbuck + u + 1]
                s = cell_slot_start[b, u]
                colp[s:s + e1 - e0] = cc[e0:e1] - u * bsz
                rowp[s:s + e1 - e0] = r[e0:e1] - b * P
                valp[s:s + e1 - e0] = vv[e0:e1]
        # idx wrap layout per call: [16, S*8] replicated to 128 partitions
        colT16 = np.zeros((P, T * 8), dtype=np.int16)
        for (u, t0, t1) in calls:
            S = t1 - t0
            lc = colp[t0 * P:t1 * P].astype(np.int16)
            colT16[:, t0 * 8:t1 * 8] = np.tile(lc.reshape(S * 8, 16).T, (8, 1))
        packed.append({
            "colT": colT16,
            "rowT": np.ascontiguousarray(rowp.reshape(T, P).T.astype(np_mdt)),
            "valT": np.ascontiguousarray(valp.reshape(T, P).T.astype(np_mdt)),
        })

    return {
        "R": R, "n_blocks": n_blocks, "nbuck": nbuck, "bsz": bsz, "T": T,
        "n_wg": n_wg, "jblk": jblk, "M_bu": M_bu, "calls": calls,
        "wg_tile_range": wg_tile_range, "tiles_b": tiles_b, "tiles_u": tiles_u,
        "first_tile": first_tile, "last_tile": last_tile,
        "packed": packed, "np_mdt": np_mdt,
    }


# --------------------------------------------------------------------------
# Bass program builder (identical program for every core; data differs)
# --------------------------------------------------------------------------

def build_program(N, L, st, use_bf16=False, n_cores=C):
    mdt = bf16 if use_bf16 else f32
    R, n_blocks, T = st["R"], st["n_blocks"], st["T"]
    jblk, n_wg, bsz = st["jblk"], st["n_wg"], st["bsz"]
    calls, wg_tile_range = st["calls"], st["wg_tile_range"]
    tiles_b, first_tile, last_tile = st["tiles_b"], st["first_tile"], st["last_tile"]
    R_pad = n_blocks * P

    nc = bacc.Bacc("TRN2", target_bir_lowering=False, debug=False,
                   num_devices=n_cores)
    x0_d = nc.dram_tensor("x0", [N, D], f32, kind="ExternalInput")
    colT_d = nc.dram_tensor("colT", [P, T * 8], i16, kind="ExternalInput")
    rowT_d = nc.dram_tensor("rowT", [P, T], mdt, kind="ExternalInput")
    valT_d = nc.dram_tensor("valT", [P, T], mdt, kind="ExternalInput")
    iota_d = nc.dram_tensor("iota", [P, P], mdt, kind="ExternalInput")
    noise_d = nc.dram_tensor("noise", [L, R_pad, D], f32, kind="ExternalInput")
    out_d = nc.dram_tensor("out", [R, D], f32, kind="ExternalOutput")

    xm = [nc.dram_tensor(f"xm{k}", [R, D], f32) for k in range(L - 1)]
    xg = [nc.dram_tensor(f"xg{k}", [N, D], f32) for k in range(1, L)]
    groups = [list(range(n_cores))]

    # map tile -> call start (for G slot indexing)
    call_of_tile = np.zeros(T, dtype=np.int64)
    for ci, (u, t0, t1) in enumerate(calls):
        call_of_tile[t0:t1] = ci

    with tile.TileContext(nc) as tc:
        with ExitStack() as ctx:
            const_p = ctx.enter_context(tc.tile_pool(name="const", bufs=1))
            ls_p = ctx.enter_context(tc.tile_pool(name="lsp", bufs=1))
            io_p = ctx.enter_context(tc.tile_pool(name="iop", bufs=3))
            g_p = ctx.enter_context(tc.tile_pool(name="gp", bufs=4))
            m_p = ctx.enter_context(tc.tile_pool(name="mp", bufs=16))
            nz_p = ctx.enter_context(tc.tile_pool(name="nzp", bufs=2))
            st_p = ctx.enter_context(tc.tile_pool(name="stp", bufs=2))
            sm_p = ctx.enter_context(tc.tile_pool(name="smp", bufs=4))
            ps = ctx.enter_context(tc.tile_pool(name="ps", bufs=8, space="PSUM"))

            iota_t = const_p.tile([P, P], mdt)
            nc.sync.dma_start(out=iota_t[:], in_=iota_d[:, :])
            ls = ls_p.tile([P, n_blocks * D], f32)

            for k in range(L):
                src = x0_d if k == 0 else xg[k - 1]
                psum_tiles = {}
                stage = {}
                nz_tiles = {}

                for wg in range(n_wg):
                    wt0, wt1 = wg_tile_range[wg]
                    wS = wt1 - wt0
                    idx_t = io_p.tile([P, wS * 8], i16, tag="idx",
                                      name=f"idx{k}_{wg}")
                    nc.sync.dma_start(out=idx_t[:],
                                      in_=colT_d[:, wt0 * 8:wt1 * 8])
                    rowv_t = io_p.tile([P, wS], mdt, tag="rowv",
                                       name=f"rowv{k}_{wg}")
                    nc.sync.dma_start(out=rowv_t[:], in_=rowT_d[:, wt0:wt1])
                    val_t = io_p.tile([P, wS], mdt, tag="val",
                                      name=f"val{k}_{wg}")
                    nc.sync.dma_start(out=val_t[:], in_=valT_d[:, wt0:wt1])

                    wg_calls = [c_ for c_ in calls if wt0 <= c_[1] < wt1]
                    g_tiles = {}
                    for (u, t0, t1) in wg_calls:
                        S = t1 - t0
                        g_t = g_p.tile([P, SCALL, D], f32, tag="g",
                                       name=f"g{k}_{t0}")
                        g_tiles[t0] = g_t
                        u1 = min((u + 1) * bsz, N)
                        nc.gpsimd.dma_gather(
                            out_ap=g_t[:, :S, :],
                            in_ap=src[u * bsz:u1, :],
                            idxs_ap=idx_t[:, (t0 - wt0) * 8:(t1 - wt0) * 8],
                            num_idxs=S * P,
                            num_idxs_reg=S * P,
                            elem_size=D,
                            single_packet=False,
                        )
                        for ti in range(t0, t1):
                            b = int(tiles_b[ti])
                            # val-weighted one-hot: (iota == row[p]) * val[p]
                            mk = m_p.tile([P, P], mdt, tag="mask",
                                          name=f"mk{k}_{ti}")
                            mask_eng = nc.vector if ti % 2 == 0 else nc.gpsimd
                            mask_eng.tensor_scalar(
                                out=mk[:], in0=iota_t[:],
                                scalar1=rowv_t[:, ti - wt0:ti - wt0 + 1],
                                scalar2=val_t[:, ti - wt0:ti - wt0 + 1],
                                op0=mybir.AluOpType.is_equal,
                                op1=mybir.AluOpType.mult)
                            if ti == first_tile[b]:
                                psum_tiles[b] = ps.tile([P, D], f32, tag="acc",
                                                        name=f"acc{k}_{b}")
                            nc.tensor.matmul(
                                out=psum_tiles[b][:],
                                lhsT=mk[:],
                                rhs=g_t[:, ti - t0, :],
                                start=(ti == first_tile[b]),
                                stop=(ti == last_tile[b]),
                            )
                            if ti != last_tile[b]:
                                continue

                            # ---- block b complete: noise step ----
                            y = psum_tiles.pop(b)
                            rows = min(P, R - b * P)
                            bwg = b // jblk
                            j0 = bwg * jblk
                            jn = min(jblk, n_blocks - j0)
                            if bwg not in nz_tiles:
                                nz = nz_p.tile([P, jblk, D], f32, tag="nz",
                                               name=f"nz{k}_{bwg}")
                                nc.scalar.dma_start(
                                    out=nz[:, :jn, :],
                                    in_=noise_d[k, j0 * P:(j0 + jn) * P, :]
                                    .rearrange("(j p) d -> p j d", p=P))
                                nz_tiles[bwg] = nz
                                if k > 0:
                                    stage[bwg] = st_p.tile(
                                        [P, jblk, D], f32, tag="st",
                                        name=f"st{k}_{bwg}")
                            nz = nz_tiles[bwg]
                            bj = b - j0
                            nzb = nz[:rows, bj, :]

                            sq = sm_p.tile([P, D], f32, tag="sq",
                                           name=f"sq{k}_{b}")
                            ss = sm_p.tile([P, 1], f32, tag="ss",
                                           name=f"ss{k}_{b}")
                            nc.scalar.activation(
                                sq[:rows, :], nzb,
                                mybir.ActivationFunctionType.Square,
                                accum_out=ss[:rows, :])
                            nc.scalar.sqrt(ss[:rows, :], ss[:rows, :])
                            nc.vector.tensor_scalar(
                                out=ss[:rows, :], in0=ss[:rows, :],
                                scalar1=NORM_EPS, scalar2=None,
                                op0=mybir.AluOpType.max)
                            rinv = sm_p.tile([P, 1], f32, tag="rinv",
                                             name=f"ri{k}_{b}")
                            nc.vector.reciprocal(rinv[:rows, :], ss[:rows, :])
                            delta = sm_p.tile([P, D], f32, tag="delta",
                                              name=f"dl{k}_{b}")
                            nc.vector.tensor_scalar(
                                out=delta[:rows, :], in0=nzb,
                                scalar1=rinv[:rows, :], scalar2=EPS,
                                op0=mybir.AluOpType.mult,
                                op1=mybir.AluOpType.mult)
                            sgn = sm_p.tile([P, D], f32, tag="sgn",
                                            name=f"sg{k}_{b}")
                            nc.scalar.sign(sgn[:rows, :], y[:rows, :])
                            nc.vector.tensor_tensor(
                                out=delta[:rows, :], in0=delta[:rows, :],
                                in1=sgn[:rows, :], op=mybir.AluOpType.mult)
                            lsb = ls[:rows, b * D:(b + 1) * D]
                            if k == 0:
                                nc.vector.tensor_tensor(
                                    out=lsb, in0=y[:rows, :],
                                    in1=delta[:rows, :],
                                    op=mybir.AluOpType.add)
                            else:
                                stb = stage[bwg][:rows, bj, :]
                                nc.vector.tensor_tensor(
                                    out=stb, in0=y[:rows, :],
                                    in1=delta[:rows, :],
                                    op=mybir.AluOpType.add)
                                nc.vector.tensor_tensor(
                                    out=lsb, in0=lsb, in1=stb,
                                    op=mybir.AluOpType.add)

                            # ---- write-group complete: DMA x rows ----
                            if k < L - 1 and b == min(j0 + jblk, n_blocks) - 1:
                                src_t = (ls[:, j0 * D:(j0 + jn) * D]
                                         .rearrange("p (j d) -> p j d", d=D)
                                         if k == 0 else stage[bwg][:, :jn, :])
                                r0 = j0 * P
                                nfull = jn if (j0 + jn) * P <= R else jn - 1
                                if nfull > 0:
                                    nc.sync.dma_start(
                                        out=xm[k][r0:r0 + nfull * P, :]
                                        .rearrange("(j p) d -> p j d", p=P),
                                        in_=src_t[:, :nfull, :])
                                if nfull < jn:
                                    tr = R - (r0 + nfull * P)
                                    nc.sync.dma_start(
                                        out=xm[k][r0 + nfull * P:R, :],
                                        in_=src_t[:tr, nfull, :])

                if k < L - 1:
                    nc.gpsimd.collective_compute(
                        "AllGather", mybir.AluOpType.bypass,
                        replica_groups=groups,
                        ins=[xm[k].ap().opt()],
                        outs=[xg[k].ap().opt()],
                    )

            # ---- final output: out = layer_sum / L ----
            for wg in range(n_wg):
                j0 = wg * JBLK if False else wg * st["jblk"]
                jn = min(st["jblk"], n_blocks - j0)
                ost = st_p.tile([P, st["jblk"], D], f32, tag="ost",
                                name=f"ost{wg}")
                nc.vector.tensor_scalar(
                    out=ost[:, :jn, :],
                    in0=ls[:, j0 * D:(j0 + jn) * D]
                    .rearrange("p (j d) -> p j d", d=D),
                    scalar1=1.0 / L, scalar2=None, op0=mybir.AluOpType.mult)
                r0 = j0 * P
                nfull = jn if (j0 + jn) * P <= R else jn - 1
                if nfull > 0:
                    nc.sync.dma_start(
                        out=out_d[r0:r0 + nfull * P, :]
                        .rearrange("(j p) d -> p j d", p=P),
                        in_=ost[:, :nfull, :])
                if nfull < jn:
                    tr = R - (r0 + nfull * P)
                    nc.sync.dma_start(out=out_d[r0 + nfull * P:R, :],
                                      in_=ost[:tr, nfull, :])
    nc.compile()
    return nc


# --------------------------------------------------------------------------
# In-maps assembly
# --------------------------------------------------------------------------

def build_in_maps(x0, noise, st, L):
    n_cores = len(st["packed"])
    R, n_blocks = st["R"], st["n_blocks"]
    R_pad = n_blocks * P
    np_mdt = st["np_mdt"]
    iota = np.tile(np.arange(P, dtype=np.float32)[None, :], (P, 1)).astype(np_mdt)
    in_maps = []
    for c in range(n_cores):
        nz = np.ones((L, R_pad, D), dtype=np.float32)
        nz[:, :R, :] = noise[:, c * R:(c + 1) * R, :]
        in_maps.append({
            "x0": x0,
            "colT": st["packed"][c]["colT"],
            "rowT": st["packed"][c]["rowT"],
            "valT": st["packed"][c]["valT"],
            "iota": iota,
            "noise": nz,
        })
    return in_maps


# --------------------------------------------------------------------------
# Entry point
# --------------------------------------------------------------------------

_CACHE = {}
USE_BF16 = False


def kernel(user_emb, item_emb, adj_row, adj_col, adj_val, noise, _trace=False):
    user_emb = np.asarray(user_emb, dtype=np.float32)
    item_emb = np.asarray(item_emb, dtype=np.float32)
    adj_row = np.asarray(adj_row).astype(np.int64)
    adj_col = np.asarray(adj_col).astype(np.int64)
    adj_val = np.asarray(adj_val, dtype=np.float32)
    noise = np.asarray(noise, dtype=np.float32)

    U = user_emb.shape[0]
    N = U + item_emb.shape[0]
    L = noise.shape[0]
    x0 = np.concatenate([user_emb, item_emb], axis=0)

    st = build_structure(adj_row, adj_col, adj_val, N, C, USE_BF16)
    key = ("prog", N, L, st["T"], USE_BF16)
    if key not in _CACHE:
        _CACHE.clear()
        _CACHE[key] = build_program(N, L, st, USE_BF16, C)
    nc = _CACHE[key]

    in_maps = build_in_maps(x0, noise, st, L)
    res = run_bass_kernel_spmd(nc, in_maps, list(range(C)), trace=_trace)
    out = np.concatenate([res.results[c]["out"] for c in range(C)], axis=0)
    if _trace:
        kernel._last_results = res
    return out[:U], out[U:]


# revision 12
# speedup vs baseline: 1.6754x; 1.6754x over previous
"""Trainium2 Bass kernel for ExpSSGL encoder (3-layer SpMM + signed-noise perturbation).

Strategy (8 NeuronCores):
  - Row-range sharding: core c owns output rows [c*R, (c+1)*R), R = N/8.
  - Edges are bucketed by (128-row destination block, 30000-node source
    bucket); each 128-edge tile is segment-summed into its block via a
    val-weighted one-hot selection matrix (tensor_scalar is_equal*mult)
    matmul accumulated in PSUM.
  - x[col] rows (256B) are fetched with gpsimd dma_gather (SWDGE, int16
    in-bucket indices, <=4096 indices per call, single_packet=False).
  - After each layer, per-core updated rows are AllGather'd so every core
    has the full [N, 64] matrix for the next layer's gather.
  - layer_sum stays resident in SBUF; output = layer_sum / L.
"""
import numpy as np
import ml_dtypes
from contextlib import ExitStack

from concourse import bass, bacc, mybir, tile
from concourse.bass_utils import run_bass_kernel_spmd

P = 128
D = 64
C = 8            # cores
EPS = 0.1
NORM_EPS = 1e-12

f32 = mybir.dt.float32
bf16 = mybir.dt.bfloat16
i16 = mybir.dt.int16

BSZ = 30000      # node bucket size (int16 index range)
SCALL = 32       # max tiles per dma_gather call (4096 indices)
JBLK = 6         # blocks per write group (PSUM has 8 banks; keep 2 spare)
MGRP = 4         # tiles per batched mask build


# --------------------------------------------------------------------------
# Host-side data prep
# --------------------------------------------------------------------------

def build_structure(row, col, val, N, n_cores=C, use_bf16=False, bsz=BSZ,
                    jblk=JBLK, scall=SCALL):
    """Pack edges per core into the shared (wg, bucket, block) tile layout."""
    R = N // n_cores
    n_blocks = (R + P - 1) // P
    nbuck = (N + bsz - 1) // bsz
    n_wg = (n_blocks + jblk - 1) // jblk
    np_mdt = ml_dtypes.bfloat16 if use_bf16 else np.float32

    per_core = []
    cnts = np.zeros((n_cores, n_blocks, nbuck), dtype=np.int64)
    for c in range(n_cores):
        sel = (row >= c * R) & (row < (c + 1) * R)
        r = (row[sel] - c * R).astype(np.int64)
        cc = col[sel].astype(np.int64)
        vv = val[sel].astype(np.float32)
        b = r // P
        u = cc // bsz
        cell = b * nbuck + u
        o = np.argsort(cell, kind="stable")
        r, cc, vv, cell = r[o], cc[o], vv[o], cell[o]
        cnts[c] = np.bincount(cell, minlength=n_blocks * nbuck).reshape(
            n_blocks, nbuck)
        per_core.append((r, cc, vv))

    M_bu = ((cnts.max(axis=0) + P - 1) // P).astype(np.int64)   # [n_blocks, nbuck]
    empty = M_bu.sum(axis=1) == 0
    M_bu[empty, 0] = 1

    # tile order: (wg, u, b); calls chunk <= scall tiles within (wg, u)
    tiles_b = []          # block id per tile
    tiles_u = []          # bucket id per tile
    calls = []            # (u, t_start, t_end)
    wg_tile_range = []    # (t_start, t_end) per wg
    t = 0
    for wg in range(n_wg):
        b0, b1 = wg * jblk, min((wg + 1) * jblk, n_blocks)
        wg_start = t
        for u in range(nbuck):
            seg_start = t
            for b in range(b0, b1):
                for _ in range(M_bu[b, u]):
                    tiles_b.append(b)
                    tiles_u.append(u)
                    t += 1
            for cs in range(seg_start, t, scall):
                calls.append((u, cs, min(cs + scall, t)))
        wg_tile_range.append((wg_start, t))
    T = t
    tiles_b = np.array(tiles_b)
    tiles_u = np.array(tiles_u)

    # slot ranges per (b, u) cell in tile order
    cell_slot_start = np.zeros((n_blocks, nbuck), dtype=np.int64)
    pos = np.zeros(1, dtype=np.int64)
    tile_cnt = 0
    for wg in range(n_wg):
        b0, b1 = wg * jblk, min((wg + 1) * jblk, n_blocks)
        for u in range(nbuck):
            for b in range(b0, b1):
                cell_slot_start[b, u] = tile_cnt * P
                tile_cnt += M_bu[b, u]
    assert tile_cnt == T

    first_tile = np.full(n_blocks, -1, dtype=np.int64)
    last_tile = np.zeros(n_blocks, dtype=np.int64)
    for ti in range(T):
        b = tiles_b[ti]
        if first_tile[b] < 0:
            first_tile[b] = ti
        last_tile[b] = ti

    packed = []
    for c in range(n_cores):
        r, cc, vv = per_core[c]
        cell = (r // P) * nbuck + (cc // bsz)
        starts = np.concatenate(
            [[0], np.cumsum(np.bincount(cell, minlength=n_blocks * nbuck))])
        colp = np.zeros(T * P, dtype=np.int64)
        rowp = np.zeros(T * P, dtype=np.float32)
        valp = np.zeros(T * P, dtype=np.float32)
        for b in range(n_blocks):
            for u in range(nbuck):
                e0, e1 = starts[b * nbuck + u], starts[b * nbuck + u + 1]
                s = cell_slot_start[b, u]
                colp[s:s + e1 - e0] = cc[e0:e1] - u * bsz
                rowp[s:s + e1 - e0] = r[e0:e1] - b * P
                valp[s:s + e1 - e0] = vv[e0:e1]
        # idx wrap layout per call: [16, S*8] replicated to 128 partitions
        colT16 = np.zeros((P, T * 8), dtype=np.int16)
        for (u, t0, t1) in calls:
            S = t1 - t0
            lc = colp[t0 * P:t1 * P].astype(np.int16)
            colT16[:, t0 * 8:t1 * 8] = np.tile(lc.reshape(S * 8, 16).T, (8, 1))
        packed.append({
            "colT": colT16,
            "rowT": np.ascontiguousarray(rowp.reshape(T, P).T.astype(np_mdt)),
            "valT": np.ascontiguousarray(valp.reshape(T, P).T.astype(np_mdt)),
        })

    return {
        "R": R, "n_blocks": n_blocks, "nbuck": nbuck, "bsz": bsz, "T": T,
        "n_wg": n_wg, "jblk": jblk, "M_bu": M_bu, "calls": calls,
        "wg_tile_range": wg_tile_range, "tiles_b": tiles_b, "tiles_u": tiles_u,
        "first_tile": first_tile, "last_tile": last_tile,
        "packed": packed, "np_mdt": np_mdt,
    }


# --------------------------------------------------------------------------
# Bass program builder (identical program for every core; data differs)
# --------------------------------------------------------------------------

def build_program(N, L, st, use_bf16=False, n_cores=C):
    mdt = bf16 if use_bf16 else f32
    R, n_blocks, T = st["R"], st["n_blocks"], st["T"]
    jblk, n_wg, bsz = st["jblk"], st["n_wg"], st["bsz"]
    calls, wg_tile_range = st["calls"], st["wg_tile_range"]
    tiles_b, first_tile, last_tile = st["tiles_b"], st["first_tile"], st["last_tile"]
    R_pad = n_blocks * P

    nc = bacc.Bacc("TRN2", target_bir_lowering=False, debug=False,
                   num_devices=n_cores)
    x0_d = nc.dram_tensor("x0", [N, D], f32, kind="ExternalInput")
    colT_d = nc.dram_tensor("colT", [P, T * 8], i16, kind="ExternalInput")
    rowT_d = nc.dram_tensor("rowT", [P, T], mdt, kind="ExternalInput")
    valT_d = nc.dram_tensor("valT", [P, T], mdt, kind="ExternalInput")
    iota_d = nc.dram_tensor("iota", [P, P], mdt, kind="ExternalInput")
    noise_d = nc.dram_tensor("noise", [L, R_pad, D], f32, kind="ExternalInput")
    out_d = nc.dram_tensor("out", [R, D], f32, kind="ExternalOutput")

    xm = [nc.dram_tensor(f"xm{k}", [R, D], f32) for k in range(L - 1)]
    xg = [nc.dram_tensor(f"xg{k}", [N, D], f32) for k in range(1, L)]
    groups = [list(range(n_cores))]

    # map tile -> call start (for G slot indexing)
    call_of_tile = np.zeros(T, dtype=np.int64)
    for ci, (u, t0, t1) in enumerate(calls):
        call_of_tile[t0:t1] = ci

    with tile.TileContext(nc) as tc:
        with ExitStack() as ctx:
            const_p = ctx.enter_context(tc.tile_pool(name="const", bufs=1))
            ls_p = ctx.enter_context(tc.tile_pool(name="lsp", bufs=1))
            io_p = ctx.enter_context(tc.tile_pool(name="iop", bufs=3))
            g_p = ctx.enter_context(tc.tile_pool(name="gp", bufs=4))
            m_p = ctx.enter_context(tc.tile_pool(name="mp", bufs=16))
            nz_p = ctx.enter_context(tc.tile_pool(name="nzp", bufs=2))
            st_p = ctx.enter_context(tc.tile_pool(name="stp", bufs=2))
            sm_p = ctx.enter_context(tc.tile_pool(name="smp", bufs=4))
            ps = ctx.enter_context(tc.tile_pool(name="ps", bufs=8, space="PSUM"))

            iota_t = const_p.tile([P, P], mdt)
            nc.sync.dma_start(out=iota_t[:], in_=iota_d[:, :])
            ls = ls_p.tile([P, n_blocks * D], f32)

            for k in range(L):
                src = x0_d if k == 0 else xg[k - 1]
                psum_tiles = {}
                stage = {}
                nz_tiles = {}

                for wg in range(n_wg):
                    wt0, wt1 = wg_tile_range[wg]
                    wS = wt1 - wt0
                    idx_t = io_p.tile([P, wS * 8], i16, tag="idx",
                                      name=f"idx{k}_{wg}")
                    nc.sync.dma_start(out=idx_t[:],
                                      in_=colT_d[:, wt0 * 8:wt1 * 8])
                    rowv_t = io_p.tile([P, wS], mdt, tag="rowv",
                                       name=f"rowv{k}_{wg}")
                    nc.sync.dma_start(out=rowv_t[:], in_=rowT_d[:, wt0:wt1])
                    val_t = io_p.tile([P, wS], mdt, tag="val",
                                      name=f"val{k}_{wg}")
                    nc.sync.dma_start(out=val_t[:], in_=valT_d[:, wt0:wt1])

                    wg_calls = [c_ for c_ in calls if wt0 <= c_[1] < wt1]
                    g_tiles = {}
                    for (u, t0, t1) in wg_calls:
                        S = t1 - t0
                        g_t = g_p.tile([P, SCALL, D], f32, tag="g",
                                       name=f"g{k}_{t0}")
                        g_tiles[t0] = g_t
                        u1 = min((u + 1) * bsz, N)
                        nc.gpsimd.dma_gather(
                            out_ap=g_t[:, :S, :],
                            in_ap=src[u * bsz:u1, :],
                            idxs_ap=idx_t[:, (t0 - wt0) * 8:(t1 - wt0) * 8],
                            num_idxs=S * P,
                            num_idxs_reg=S * P,
                            elem_size=D,
                            single_packet=False,
                        )
                        # batched val fold over the whole call (gather is
                        # call-granular anyway, so this adds no latency)
                        nc.vector.tensor_tensor(
                            out=g_t[:, :S, :], in0=g_t[:, :S, :],
                            in1=val_t[:, t0 - wt0:t1 - wt0, None]
                            .to_broadcast([P, S, D]),
                            op=mybir.AluOpType.mult)
                        # one-hot masks in groups of MGRP tiles: halves the
                        # DVE instruction stream vs per-tile, but keeps the
                        # PE fed at 4-matmul granularity
                        mk4 = {}
                        for a in range(t0, t1, MGRP):
                            g = min(MGRP, t1 - a)
                            mk = m_p.tile([P, MGRP, P], mdt, tag="mask",
                                          name=f"mk{k}_{a}")
                            nc.vector.tensor_tensor(
                                out=mk[:, :g, :],
                                in0=rowv_t[:, a - wt0:a - wt0 + g, None]
                                .to_broadcast([P, g, P]),
                                in1=iota_t[:, None, :].to_broadcast([P, g, P]),
                                op=mybir.AluOpType.is_equal)
                            mk4[a] = mk
                        for ti in range(t0, t1):
                            b = int(tiles_b[ti])
                            a = t0 + ((ti - t0) // MGRP) * MGRP
                            if ti == first_tile[b]:
                                psum_tiles[b] = ps.tile([P, D], f32, tag="acc",
                                                        name=f"acc{k}_{b}")
                            nc.tensor.matmul(
                                out=psum_tiles[b][:],
                                lhsT=mk4[a][:, ti - a, :],
                                rhs=g_t[:, ti - t0, :],
                                start=(ti == first_tile[b]),
                                stop=(ti == last_tile[b]),
                            )
                            if ti != last_tile[b]:
                                continue

                            # ---- block b complete: noise step ----
                            y = psum_tiles.pop(b)
                            rows = min(P, R - b * P)
                            bwg = b // jblk
                            j0 = bwg * jblk
                            jn = min(jblk, n_blocks - j0)
                            if bwg not in nz_tiles:
                                nz = nz_p.tile([P, jblk, D], f32, tag="nz",
                                               name=f"nz{k}_{bwg}")
                                nc.scalar.dma_start(
                                    out=nz[:, :jn, :],
                                    in_=noise_d[k, j0 * P:(j0 + jn) * P, :]
                                    .rearrange("(j p) d -> p j d", p=P))
                                nz_tiles[bwg] = nz
                                if k > 0:
                                    stage[bwg] = st_p.tile(
                                        [P, jblk, D], f32, tag="st",
                                        name=f"st{k}_{bwg}")
                            nz = nz_tiles[bwg]
                            bj = b - j0
                            nzb = nz[:rows, bj, :]

                            sq = sm_p.tile([P, D], f32, tag="sq",
                                           name=f"sq{k}_{b}")
                            ss = sm_p.tile([P, 1], f32, tag="ss",
                                           name=f"ss{k}_{b}")
                            nc.scalar.activation(
                                sq[:rows, :], nzb,
                                mybir.ActivationFunctionType.Square,
                                accum_out=ss[:rows, :])
                            nc.scalar.sqrt(ss[:rows, :], ss[:rows, :])
                            nc.vector.tensor_scalar(
                                out=ss[:rows, :], in0=ss[:rows, :],
                                scalar1=NORM_EPS, scalar2=None,
                                op0=mybir.AluOpType.max)
                            rinv = sm_p.tile([P, 1], f32, tag="rinv",
                                             name=f"ri{k}_{b}")
                            nc.vector.reciprocal(rinv[:rows, :], ss[:rows, :])
                            delta = sm_p.tile([P, D], f32, tag="delta",
                                              name=f"dl{k}_{b}")
                            nc.vector.tensor_scalar(
                                out=delta[:rows, :], in0=nzb,
                                scalar1=rinv[:rows, :], scalar2=EPS,
                                op0=mybir.AluOpType.mult,
                                op1=mybir.AluOpType.mult)
                            sgn = sm_p.tile([P, D], f32, tag="sgn",
                                            name=f"sg{k}_{b}")
                            nc.scalar.sign(sgn[:rows, :], y[:rows, :])
                            nc.vector.tensor_tensor(
                                out=delta[:rows, :], in0=delta[:rows, :],
                                in1=sgn[:rows, :], op=mybir.AluOpType.mult)
                            lsb = ls[:rows, b * D:(b + 1) * D]
                            if k == 0:
                                nc.vector.tensor_tensor(
                                    out=lsb, in0=y[:rows, :],
                                    in1=delta[:rows, :],
                                    op=mybir.AluOpType.add)
                            else:
                                stb = stage[bwg][:rows, bj, :]
                                nc.vector.tensor_tensor(
                                    out=stb, in0=y[:rows, :],
                                    in1=delta[:rows, :],
                                    op=mybir.AluOpType.add)
                                nc.vector.tensor_tensor(
                                    out=lsb, in0=lsb, in1=stb,
                                    op=mybir.AluOpType.add)

                            # ---- write-group complete: DMA x rows ----
                            if k < L - 1 and b == min(j0 + jblk, n_blocks) - 1:
                                src_t = (ls[:, j0 * D:(j0 + jn) * D]
                                         .rearrange("p (j d) -> p j d", d=D)
                                         if k == 0 else stage[bwg][:, :jn, :])
                                r0 = j0 * P
                                nfull = jn if (j0 + jn) * P <= R else jn - 1
                                if nfull > 0:
                                    nc.sync.dma_start(
                                        out=xm[k][r0:r0 + nfull * P, :]
                                        .rearrange("(j p) d -> p j d", p=P),
                                        in_=src_t[:, :nfull, :])
                                if nfull < jn:
                                    tr = R - (r0 + nfull * P)
                                    nc.sync.dma_start(
                                        out=xm[k][r0 + nfull * P:R, :],
                                        in_=src_t[:tr, nfull, :])

                if k < L - 1:
                    nc.gpsimd.collective_compute(
                        "AllGather", mybir.AluOpType.bypass,
                        replica_groups=groups,
                        ins=[xm[k].ap().opt()],
                        outs=[xg[k].ap().opt()],
                    )

            # ---- final output: out = layer_sum / L ----
            for wg in range(n_wg):
                j0 = wg * JBLK if False else wg * st["jblk"]
                jn = min(st["jblk"], n_blocks - j0)
                ost = st_p.tile([P, st["jblk"], D], f32, tag="ost",
                                name=f"ost{wg}")
                nc.vector.tensor_scalar(
                    out=ost[:, :jn, :],
                    in0=ls[:, j0 * D:(j0 + jn) * D]
                    .rearrange("p (j d) -> p j d", d=D),
                    scalar1=1.0 / L, scalar2=None, op0=mybir.AluOpType.mult)
                r0 = j0 * P
                nfull = jn if (j0 + jn) * P <= R else jn - 1
                if nfull > 0:
                    nc.sync.dma_start(
                        out=out_d[r0:r0 + nfull * P, :]
                        .rearrange("(j p) d -> p j d", p=P),
                        in_=ost[:, :nfull, :])
                if nfull < jn:
                    tr = R - (r0 + nfull * P)
                    nc.sync.dma_start(out=out_d[r0 + nfull * P:R, :],
                                      in_=ost[:tr, nfull, :])
    nc.compile()
    return nc


# --------------------------------------------------------------------------
# In-maps assembly
# --------------------------------------------------------------------------

def build_in_maps(x0, noise, st, L):
    n_cores = len(st["packed"])
    R, n_blocks = st["R"], st["n_blocks"]
    R_pad = n_blocks * P
    np_mdt = st["np_mdt"]
    iota = np.tile(np.arange(P, dtype=np.float32)[None, :], (P, 1)).astype(np_mdt)
    in_maps = []
    for c in range(n_cores):
        nz = np.ones((L, R_pad, D), dtype=np.float32)
        nz[:, :R, :] = noise[:, c * R:(c + 1) * R, :]
        in_maps.append({
            "x0": x0,
            "colT": st["packed"][c]["colT"],
            "rowT": st["packed"][c]["rowT"],
            "valT": st["packed"][c]["valT"],
            "iota": iota,
            "noise": nz,
        })
    return in_maps


# --------------------------------------------------------------------------
# Entry point
# --------------------------------------------------------------------------

_CACHE = {}
USE_BF16 = False


def kernel(user_emb, item_emb, adj_row, adj_col, adj_val, noise, _trace=False):
    user_emb = np.asarray(user_emb, dtype=np.float32)
    item_emb = np.asarray(item_emb, dtype=np.float32)
    adj_row = np.asarray(adj_row).astype(np.int64)
    adj_col = np.asarray(adj_col).astype(np.int64)
    adj_val = np.asarray(adj_val, dtype=np.float32)
    noise = np.asarray(noise, dtype=np.float32)

    U = user_emb.shape[0]
    N = U + item_emb.shape[0]
    L = noise.shape[0]
    x0 = np.concatenate([user_emb, item_emb], axis=0)

    st = build_structure(adj_row, adj_col, adj_val, N, C, USE_BF16)
    key = ("prog", N, L, st["T"], USE_BF16)
    if key not in _CACHE:
        _CACHE.clear()
        _CACHE[key] = build_program(N, L, st, USE_BF16, C)
    nc = _CACHE[key]

    in_maps = build_in_maps(x0, noise, st, L)
    res = run_bass_kernel_spmd(nc, in_maps, list(range(C)), trace=_trace)
    out = np.concatenate([res.results[c]["out"] for c in range(C)], axis=0)
    if _trace:
        kernel._last_results = res
    return out[:U], out[U:]
